# revision 60
# baseline (speedup 1.0000x reference)
"""Trainium2 Bass kernel for the ACSL multi-snippet classification loss.

Algorithm (derived from the reference):
  loss = sum_{i,c} wm_last[i,c] * cls_loss[i,c] / (n_i*T)
  cls_loss[i,c] = sum_t softplus(lg[i,c,t]) - sum_t [c == argmax_c' lb[i,c',t]] * lg[i,c,t]
  wm_last depends only on snippet t=99 plus fixed (input-independent) jax randomness.

Device does the O(N) work (reads both full tensors once):
  - sp_sum[i,c] = sum_t softplus(lg[i,c,t])           (ScalarE softplus + DVE reduce)
  - keymax[i,t] = max_c ( fp16(lb*K) + (200-c)*2^-25 )  packed value+index argmax
    (ScalarE fp16 quantize, GpSimd index-pack add, DVE reduce)
Host does the tiny [1024,201]-scale finalization: index extraction from keymax,
argmax-gather subtraction, last-snippet weight mask, final weighted sum.

Sharding: data-parallel over rows (n_i axis), 128 rows per core across 8 cores.
"""

import numpy as np

N_ROWS = 1024
N_C = 201
NUM_CLASSES = 200
T = 100
N_CORES = 8
P = N_ROWS // N_CORES  # 128 rows per core == SBUF partitions
SCORE_THR = 0.3
# Argmax via packed single reduce: key = (lb AND 0xFFFFF000) OR (201-c).
# The AND floor-quantizes the (positive) label to 11 explicit mantissa bits
# (order-preserving); the OR writes the class code into the zeroed low 12
# bits (ties resolve toward smaller c, matching argmax). DVE tensor_scalar
# does the AND (2x single-src mode); gpsimd tensor_tensor float-ADDs
# (201-c)*2^-24, which lands exactly in the zeroed low bits (the AND also
# guarantees headroom, so no binade crossing); one DVE max-reduce gets
# value+index; c' = low12 * ulp(keymax) * 2^24 exactly.
QMASK_BITS = np.uint32(0xFFFFF000)
IDX_LSB = 2.0 ** -24
# small first chunks shorten the pipeline head (first-DMA latency)
CHUNKS = [15, 31, 31, 31, 31, 31, 31]  # class-axis chunks, logits side
T_CHUNKS = [10, 15, 15, 15, 15, 15, 15]  # time-axis chunks for the labels side

_CACHE = {}


def _patch_act_tables():
    """Prefer the table set containing BOTH exp and ln so the per-chunk
    Exp->Ln sequence needs one ACT_TABLE_LOAD total instead of 14."""
    from concourse import bacc as bacc_mod

    orig = bacc_mod.get_activation_tables
    if getattr(orig, "_patched_for_ln_exp", False):
        return

    def patched(arch):
        # Dict order IS the act_func_set_id wired into the NEFF, so it must
        # not change. Instead remove Exp/Ln from every other set so the
        # table chooser can only satisfy them from the combined set.
        from concourse import mybir

        t = dict(orig(arch))
        pref = "natural_log_exp_and_others"
        if pref in t:
            both = {
                mybir.ActivationFunctionType.Exp,
                mybir.ActivationFunctionType.Ln,
            }
            t = {
                k: (v if k == pref else set(v) - both) for k, v in t.items()
            }
        return t

    patched._patched_for_ln_exp = True
    bacc_mod.get_activation_tables = patched


def _build():
    """Build + compile the per-core Bass program (same SPMD program on all 8)."""
    from contextlib import ExitStack
    from concourse import bacc, mybir, tile

    _patch_act_tables()
    nc = bacc.Bacc(
        "TRN2", target_bir_lowering=False, debug=False, num_devices=N_CORES
    )
    f32 = mybir.dt.float32
    f16 = mybir.dt.float16
    AF = mybir.ActivationFunctionType
    ALU = mybir.AluOpType
    AX = mybir.AxisListType

    lg_ext = nc.dram_tensor("lg", [P, N_C, T], f32, kind="ExternalInput").ap()
    # labels arrive host-transposed to [P, T, N_C] so every class-axis op on
    # the device reads/writes with a contiguous inner axis
    lb_ext = nc.dram_tensor("lb", [P, T, N_C], f32, kind="ExternalInput").ap()
    ik_ext = nc.dram_tensor("idxk", [P, N_C], f32, kind="ExternalInput").ap()
    mask_ext = nc.dram_tensor("qmask", [P, 1], f32, kind="ExternalInput").ap()
    out_ext = nc.dram_tensor("out", [P, N_C + T], f32, kind="ExternalOutput").ap()

    with tile.TileContext(nc) as tc, ExitStack() as ctx:
        const_pool = ctx.enter_context(tc.tile_pool(name="const", bufs=1))
        lb_pool = ctx.enter_context(tc.tile_pool(name="lbp", bufs=7))
        lg_pool = ctx.enter_context(tc.tile_pool(name="lgp", bufs=4))
        key_pool = ctx.enter_context(tc.tile_pool(name="keyp", bufs=3))
        acc_pool = ctx.enter_context(tc.tile_pool(name="accp", bufs=1))

        ik = const_pool.tile([P, N_C], f32)
        nc.sync.dma_start(out=ik[:], in_=ik_ext[:])
        qmask = const_pool.tile([P, 1], f32)
        nc.sync.dma_start(out=qmask[:], in_=mask_ext[:])

        sp_out = acc_pool.tile([P, N_C], f32)
        keymax = acc_pool.tile([P, T], f32)

        # per-chunk offsets
        lg_off = []
        c0 = 0
        for cc in CHUNKS:
            lg_off.append((c0, cc))
            c0 += cc
        lb_off = []
        t0 = 0
        for tc_sz in T_CHUNKS:
            lb_off.append((t0, tc_sz))
            t0 += tc_sz

        tlg_t, tsp_t, tlb_t, tq_t, tkey_t = {}, {}, {}, {}, {}

        # stage emitters: engines execute their queues in emission order, so
        # the global sequence below is a hand-crafted static schedule
        def dma_lg(i):
            c0, cc = lg_off[i]
            tlg_t[i] = lg_pool.tile([P, cc * T], f32, tag="lg", name=f"tlg{i}")
            nc.sync.dma_start(
                out=tlg_t[i][:].rearrange("p (c t) -> p c t", t=T),
                in_=lg_ext[:, c0 : c0 + cc, :],
            )

        def dma_lb(j):
            t0, tc_sz = lb_off[j]
            tlb_t[j] = lb_pool.tile([P, tc_sz * N_C], f32, tag="lb", name=f"tlb{j}")
            nc.sync.dma_start(
                out=tlb_t[j][:].rearrange("p (t c) -> p t c", c=N_C),
                in_=lb_ext[:, t0 : t0 + tc_sz, :],
            )

        def exp_ln(i):
            # softplus fully in place on the logits tile
            nc.scalar.activation(tlg_t[i][:], tlg_t[i][:], AF.Exp)
            nc.scalar.activation(tlg_t[i][:], tlg_t[i][:], AF.Ln, bias=1.0)

        def sp_red(i):
            c0, cc = lg_off[i]
            nc.vector.tensor_reduce(
                out=sp_out[:, c0 : c0 + cc],
                in_=tlg_t[i][:].rearrange("p (c t) -> p c t", t=T),
                axis=AX.X,
                op=ALU.add,
            )

        def quant(j):
            # lb &= qmask in place: floor-quantize, DVE 2x single-src mode
            nc.vector.tensor_scalar(
                out=tlb_t[j][:].bitcast(mybir.dt.uint32),
                in0=tlb_t[j][:].bitcast(mybir.dt.uint32),
                scalar1=qmask[:].bitcast(mybir.dt.uint32),
                scalar2=None,
                op0=ALU.bitwise_and,
            )

        def key_tt(j):
            # key = q OR classcode on gpsimd
            t0, tc_sz = lb_off[j]
            tkey_t[j] = key_pool.tile([P, tc_sz * N_C], f32, tag="key", name=f"tkey{j}")
            nc.gpsimd.tensor_tensor(
                out=tkey_t[j][:].rearrange("p (t c) -> p t c", c=N_C),
                in0=tlb_t[j][:].rearrange("p (t c) -> p t c", c=N_C),
                in1=ik[:].unsqueeze(1).broadcast_to([P, tc_sz, N_C]),
                op=ALU.add,
            )

        def key_red(j):
            t0, tc_sz = lb_off[j]
            nc.vector.tensor_reduce(
                out=keymax[:, t0 : t0 + tc_sz],
                in_=tkey_t[j][:].rearrange("p (t c) -> p t c", c=N_C),
                axis=AX.X,
                op=ALU.max,
            )

        ops = {
            "Dg": dma_lg, "Db": dma_lb, "A": exp_ln, "S": sp_red,
            "Q": quant, "K": key_tt, "R": key_red,
        }
        # hand schedule; each engine executes its projection in emission
        # order. The labels side feeds the longest dependency chain
        # (DMA->Q->TT->R), so its DMAs are front-loaded; logits DMAs trickle
        # early to keep ACT fed, then take the remaining bandwidth. Qs run as
        # early as their data lands; Rs lag so they never head-of-line-block
        # the DVE queue.
        schedule = [
            "Dg0", "Db0", "Db1", "A0", "Q0", "K0", "Q1", "Db2", "Dg1",
            "S0", "Q2", "K1", "A1", "Db3", "Q3", "K2", "S1", "Db4",
            "Dg2", "A2", "Q4", "K3", "R0", "Db5", "S2", "Q5", "K4",
            "Dg3", "A3", "R1", "Db6", "S3", "Q6", "K5", "R2",
            "Dg4", "A4", "S4", "K6", "R3",
            "Dg5", "A5", "S5", "R4",
            "Dg6", "A6", "S6", "R5", "R6",
        ]
        for item in schedule:
            kind = "".join(ch for ch in item if not ch.isdigit())
            ops[kind](int(item[len(kind):]))

        nc.sync.dma_start(out=out_ext[:, 0:N_C], in_=sp_out[:])
        nc.sync.dma_start(out=out_ext[:, N_C : N_C + T], in_=keymax[:])

    nc.compile()
    return nc


def _get_nc():
    if "nc" not in _CACHE:
        _CACHE["nc"] = _build()
    return _CACHE["nc"]


def run_device(lg, lb, trace=False, **kw):
    """Run the SPMD device program. Returns (sp_sum[1024,201], keymax[1024,100], results)."""
    from concourse.bass_utils import run_bass_kernel_spmd

    nc = _get_nc()
    idxk = ((NUM_CLASSES + 1 - np.arange(N_C)) * IDX_LSB).astype(np.float32)
    ik_tile = np.ascontiguousarray(np.broadcast_to(idxk, (P, N_C)))
    mask_tile = np.ascontiguousarray(
        np.broadcast_to(QMASK_BITS.view(np.float32), (P, 1))
    )
    lbT = np.ascontiguousarray(lb.transpose(0, 2, 1))  # [rows, T, N_C]
    in_maps = []
    for core in range(N_CORES):
        r0 = core * P
        in_maps.append(
            {
                "lg": np.ascontiguousarray(lg[r0 : r0 + P]),
                "lb": lbT[r0 : r0 + P],
                "idxk": ik_tile,
                "qmask": mask_tile,
            }
        )
    res = run_bass_kernel_spmd(
        nc, in_maps, core_ids=list(range(N_CORES)), trace=trace, **kw
    )
    out_full = np.concatenate(
        [np.asarray(res.results[i]["out"]) for i in range(N_CORES)], axis=0
    )
    return out_full[:, :N_C], out_full[:, N_C:], res


def _host_finalize(lg, lb, sp_sum, keymax):
    """Tiny [1024,201]-scale finalization mirroring the reference semantics."""
    import jax
    import jax.numpy as jnp

    # --- extract per-(i,t) argmax class from the packed keymax ---
    # low 12 bits hold (201-c) in units of 2^-24/ulp(keymax), exactly
    kb = np.ascontiguousarray(keymax).view(np.uint32)
    exp = ((kb >> 23) & 0xFF).astype(np.int64)
    low12 = (kb & 0xFFF).astype(np.int64)
    scale = np.exp2((exp - 127 - 23 + 24).astype(np.float64))
    cprime = np.rint(low12 * scale).astype(np.int64)
    idx = (NUM_CLASSES + 1) - cprime
    np.clip(idx, 0, NUM_CLASSES, out=idx)

    # --- cls_loss = sp_sum - scatter-subtract of gathered logits ---
    ii = np.arange(N_ROWS)[:, None]
    tt = np.arange(T)[None, :]
    g = lg[ii, idx, tt].astype(np.float64)
    cls_loss = sp_sum.astype(np.float64).copy()
    np.add.at(cls_loss, (ii, idx), -g)

    # --- last-snippet weight mask (exact reference semantics) ---
    lg99 = lg[:, :, T - 1]
    lb99 = lb[:, :, T - 1]
    labels99 = lb99.argmax(axis=1)
    is_bg = labels99 == NUM_CLASSES
    n_bg = int(is_bg.sum())

    cpu = jax.devices("cpu")[0]
    with jax.default_device(cpu):
        keys = jax.random.split(jax.random.key(42), T)
        k1, k2 = jax.random.split(keys[T - 1])
        u1 = np.asarray(jax.random.uniform(k1, (N_ROWS,)))
        u2 = np.asarray(jax.random.uniform(k2, (N_ROWS,)))
        score_mask = np.asarray(jax.nn.sigmoid(jnp.asarray(lg99))) >= np.float32(
            SCORE_THR
        )

    def _sel(u, m):
        um = np.where(is_bg, u, np.inf).astype(np.float32)
        order = np.argsort(um, kind="stable")
        ranks = np.zeros(N_ROWS, np.int64)
        ranks[order] = np.arange(N_ROWS)
        return is_bg & (ranks < m)

    sel_rare = _sel(u1, n_bg // 100)
    sel_common = _sel(u2, n_bg // 10)

    cls_id = np.arange(N_C)
    rare_m = (cls_id < 50).astype(np.float64)
    common_m = ((cls_id >= 50) & (cls_id < 150)).astype(np.float64)
    freq_m = ((cls_id >= 150) & (cls_id < 200)).astype(np.float64)
    bg_col = (cls_id == NUM_CLASSES).astype(np.float64)

    target99 = (labels99[:, None] == cls_id[None, :]).astype(np.float64)
    wm = np.where(is_bg[:, None], 0.0, score_mask.astype(np.float64))
    ind = (
        target99
        + is_bg[:, None] * (freq_m + bg_col)[None, :]
        + sel_rare[:, None] * rare_m[None, :]
        + sel_common[:, None] * common_m[None, :]
    )
    wm = np.maximum(wm, np.clip(ind, 0.0, 1.0))

    loss = (wm * cls_loss).sum() / (N_ROWS * T)
    return np.array(loss, dtype=np.float32)


def kernel(cls_logits_, labels_):
    lg = np.ascontiguousarray(np.asarray(cls_logits_, dtype=np.float32))
    lb = np.ascontiguousarray(np.asarray(labels_, dtype=np.float32))
    sp_sum, keymax, _ = run_device(lg, lb, trace=False)
    return _host_finalize(lg, lb, sp_sum, keymax)


# revision 67
# speedup vs baseline: 1.1755x; 1.1755x over previous
"""Trainium2 Bass kernel for the ACSL multi-snippet classification loss.

Algorithm (derived from the reference):
  loss = sum_{i,c} wm_last[i,c] * cls_loss[i,c] / (n_i*T)
  cls_loss[i,c] = sum_t softplus(lg[i,c,t]) - sum_t [c == argmax_c' lb[i,c',t]] * lg[i,c,t]
  wm_last depends only on snippet t=99 plus fixed (input-independent) jax randomness.

Device does the O(N) work (reads both full tensors once):
  - sp_sum[i,c] = sum_t softplus(lg[i,c,t])
    (ScalarE Exp then Ln(x+1) in place, DVE sum-reduce over t)
  - keymax[i,t] = max_c ( (lb[i,c,t] AND 0xFFFFF000) + (201-c)*2^-24 )
    packed value+index argmax (DVE bitwise-AND quantize, GpSimd index add,
    DVE max-reduce over c)
Host does the tiny [1024,201]-scale finalization: index extraction from keymax,
argmax-gather subtraction, last-snippet weight mask, final weighted sum.

Sharding: data-parallel over rows (n_i axis), 128 rows per core across 8 cores.
labels_ is transposed to [rows, T, N_C] during input sharding so every
class-axis op on the device has a contiguous inner axis.
"""

import numpy as np

N_ROWS = 1024
N_C = 201
NUM_CLASSES = 200
T = 100
N_CORES = 8
P = N_ROWS // N_CORES  # 128 rows per core == SBUF partitions
SCORE_THR = 0.3
# Argmax via packed single reduce: key = (lb AND 0xFFFFF000) OR (201-c).
# The AND floor-quantizes the (positive) label to 11 explicit mantissa bits
# (order-preserving); the OR writes the class code into the zeroed low 12
# bits (ties resolve toward smaller c, matching argmax). DVE tensor_scalar
# does the AND (2x single-src mode); gpsimd tensor_tensor float-ADDs
# (201-c)*2^-24, which lands exactly in the zeroed low bits (the AND also
# guarantees headroom, so no binade crossing); one DVE max-reduce gets
# value+index; c' = low12 * ulp(keymax) * 2^24 exactly.
QMASK_BITS = np.uint32(0xFFFFF000)
IDX_LSB = 2.0 ** -24
# small first chunks shorten the pipeline head (first-DMA latency)
CHUNKS = [15, 31, 31, 31, 31, 31, 31]  # class-axis chunks, logits side
T_CHUNKS = [10, 15, 15, 15, 15, 15, 15]  # time-axis chunks for the labels side
LB_BUFS = 7
LG_BUFS = 4
KEY_BUFS = 3

_CACHE = {}


def _patch_act_tables():
    """Prefer the table set containing BOTH exp and ln so the per-chunk
    Exp->Ln sequence needs one ACT_TABLE_LOAD total instead of 14."""
    from concourse import bacc as bacc_mod

    orig = bacc_mod.get_activation_tables
    if getattr(orig, "_patched_for_ln_exp", False):
        return

    def patched(arch):
        # Dict order IS the act_func_set_id wired into the NEFF, so it must
        # not change. Instead remove Exp/Ln from every other set so the
        # table chooser can only satisfy them from the combined set.
        from concourse import mybir

        t = dict(orig(arch))
        pref = "natural_log_exp_and_others"
        if pref in t:
            both = {
                mybir.ActivationFunctionType.Exp,
                mybir.ActivationFunctionType.Ln,
            }
            t = {
                k: (v if k == pref else set(v) - both) for k, v in t.items()
            }
        return t

    patched._patched_for_ln_exp = True
    bacc_mod.get_activation_tables = patched


def _build():
    """Build + compile the per-core Bass program (same SPMD program on all 8)."""
    from contextlib import ExitStack
    from concourse import bacc, mybir, tile

    _patch_act_tables()
    nc = bacc.Bacc(
        "TRN2", target_bir_lowering=False, debug=False, num_devices=N_CORES
    )
    f32 = mybir.dt.float32
    AF = mybir.ActivationFunctionType
    ALU = mybir.AluOpType
    AX = mybir.AxisListType

    lg_ext = nc.dram_tensor("lg", [P, N_C, T], f32, kind="ExternalInput").ap()
    # labels arrive host-transposed to [P, T, N_C] so every class-axis op on
    # the device reads/writes with a contiguous inner axis
    lb_ext = nc.dram_tensor("lb", [P, T, N_C], f32, kind="ExternalInput").ap()
    ik_ext = nc.dram_tensor("idxk", [P, N_C], f32, kind="ExternalInput").ap()
    mask_ext = nc.dram_tensor("qmask", [P, 1], f32, kind="ExternalInput").ap()
    out_ext = nc.dram_tensor("out", [P, N_C + T], f32, kind="ExternalOutput").ap()

    with tile.TileContext(nc) as tc, ExitStack() as ctx:
        const_pool = ctx.enter_context(tc.tile_pool(name="const", bufs=1))
        lb_pool = ctx.enter_context(tc.tile_pool(name="lbp", bufs=LB_BUFS))
        lg_pool = ctx.enter_context(tc.tile_pool(name="lgp", bufs=LG_BUFS))
        key_pool = ctx.enter_context(tc.tile_pool(name="keyp", bufs=KEY_BUFS))
        acc_pool = ctx.enter_context(tc.tile_pool(name="accp", bufs=1))

        ik = const_pool.tile([P, N_C], f32)
        nc.sync.dma_start(out=ik[:], in_=ik_ext[:])
        qmask = const_pool.tile([P, 1], f32)
        nc.sync.dma_start(out=qmask[:], in_=mask_ext[:])

        sp_out = acc_pool.tile([P, N_C], f32)
        keymax = acc_pool.tile([P, T], f32)

        # per-chunk offsets
        lg_off = []
        c0 = 0
        for cc in CHUNKS:
            lg_off.append((c0, cc))
            c0 += cc
        lb_off = []
        t0 = 0
        for tc_sz in T_CHUNKS:
            lb_off.append((t0, tc_sz))
            t0 += tc_sz

        tlg_t, tlb_t, tkey_t = {}, {}, {}

        # stage emitters: engines execute their queues in emission order, so
        # the global sequence below is a hand-crafted static schedule
        def dma_lg(i):
            c0, cc = lg_off[i]
            tlg_t[i] = lg_pool.tile([P, cc * T], f32, tag="lg", name=f"tlg{i}")
            nc.sync.dma_start(
                out=tlg_t[i][:].rearrange("p (c t) -> p c t", t=T),
                in_=lg_ext[:, c0 : c0 + cc, :],
            )

        def dma_lb(j):
            t0, tc_sz = lb_off[j]
            tlb_t[j] = lb_pool.tile([P, tc_sz * N_C], f32, tag="lb", name=f"tlb{j}")
            nc.sync.dma_start(
                out=tlb_t[j][:].rearrange("p (t c) -> p t c", c=N_C),
                in_=lb_ext[:, t0 : t0 + tc_sz, :],
            )

        def exp_ln(i):
            # softplus fully in place on the logits tile
            nc.scalar.activation(tlg_t[i][:], tlg_t[i][:], AF.Exp)
            nc.scalar.activation(tlg_t[i][:], tlg_t[i][:], AF.Ln, bias=1.0)

        def sp_red(i):
            c0, cc = lg_off[i]
            nc.vector.tensor_reduce(
                out=sp_out[:, c0 : c0 + cc],
                in_=tlg_t[i][:].rearrange("p (c t) -> p c t", t=T),
                axis=AX.X,
                op=ALU.add,
            )

        def quant(j):
            # lb &= qmask in place: floor-quantize, DVE 2x single-src mode
            nc.vector.tensor_scalar(
                out=tlb_t[j][:].bitcast(mybir.dt.uint32),
                in0=tlb_t[j][:].bitcast(mybir.dt.uint32),
                scalar1=qmask[:].bitcast(mybir.dt.uint32),
                scalar2=None,
                op0=ALU.bitwise_and,
            )

        def key_tt(j):
            # key = q OR classcode on gpsimd
            t0, tc_sz = lb_off[j]
            tkey_t[j] = key_pool.tile([P, tc_sz * N_C], f32, tag="key", name=f"tkey{j}")
            nc.gpsimd.tensor_tensor(
                out=tkey_t[j][:].rearrange("p (t c) -> p t c", c=N_C),
                in0=tlb_t[j][:].rearrange("p (t c) -> p t c", c=N_C),
                in1=ik[:].unsqueeze(1).broadcast_to([P, tc_sz, N_C]),
                op=ALU.add,
            )

        def key_red(j):
            t0, tc_sz = lb_off[j]
            nc.vector.tensor_reduce(
                out=keymax[:, t0 : t0 + tc_sz],
                in_=tkey_t[j][:].rearrange("p (t c) -> p t c", c=N_C),
                axis=AX.X,
                op=ALU.max,
            )

        ops = {
            "Dg": dma_lg, "Db": dma_lb, "A": exp_ln, "S": sp_red,
            "Q": quant, "K": key_tt, "R": key_red,
        }
        # hand schedule; each engine executes its projection in emission
        # order. The labels side feeds the longest dependency chain
        # (DMA->Q->TT->R), so its DMAs are front-loaded; logits DMAs trickle
        # early to keep ACT fed, then take the remaining bandwidth. Qs run as
        # early as their data lands; Rs lag so they never head-of-line-block
        # the DVE queue.
        schedule = [
            "Dg0", "Db0", "Db1", "A0", "Q0", "K0", "Q1", "Db2", "Dg1",
            "S0", "Q2", "K1", "A1", "Db3", "Q3", "K2", "S1", "Db4",
            "Dg2", "A2", "Q4", "K3", "R0", "Db5", "S2", "Q5", "K4",
            "Dg3", "A3", "R1", "Db6", "S3", "Q6", "K5", "R2",
            "Dg4", "A4", "S4", "K6", "R3",
            "Dg5", "A5", "S5", "R4",
            "Dg6", "A6", "S6", "R5", "R6",
        ]
        for item in schedule:
            kind = "".join(ch for ch in item if not ch.isdigit())
            ops[kind](int(item[len(kind):]))

        nc.sync.dma_start(out=out_ext[:, 0:N_C], in_=sp_out[:])
        nc.sync.dma_start(out=out_ext[:, N_C : N_C + T], in_=keymax[:])

    nc.compile()
    return nc


def _get_nc():
    if "nc" not in _CACHE:
        _CACHE["nc"] = _build()
    return _CACHE["nc"]


def run_device(lg, lb, trace=False, **kw):
    """Run the SPMD device program. Returns (sp_sum[1024,201], keymax[1024,100], results)."""
    from concourse.bass_utils import run_bass_kernel_spmd

    nc = _get_nc()
    idxk = ((NUM_CLASSES + 1 - np.arange(N_C)) * IDX_LSB).astype(np.float32)
    ik_tile = np.ascontiguousarray(np.broadcast_to(idxk, (P, N_C)))
    mask_tile = np.ascontiguousarray(
        np.broadcast_to(QMASK_BITS.view(np.float32), (P, 1))
    )
    lbT = np.ascontiguousarray(lb.transpose(0, 2, 1))  # [rows, T, N_C]
    in_maps = []
    for core in range(N_CORES):
        r0 = core * P
        in_maps.append(
            {
                "lg": np.ascontiguousarray(lg[r0 : r0 + P]),
                "lb": lbT[r0 : r0 + P],
                "idxk": ik_tile,
                "qmask": mask_tile,
            }
        )
    res = run_bass_kernel_spmd(
        nc, in_maps, core_ids=list(range(N_CORES)), trace=trace, **kw
    )
    out_full = np.concatenate(
        [np.asarray(res.results[i]["out"]) for i in range(N_CORES)], axis=0
    )
    return out_full[:, :N_C], out_full[:, N_C:], res


def _host_finalize(lg, lb, sp_sum, keymax):
    """Tiny [1024,201]-scale finalization mirroring the reference semantics."""
    import jax
    import jax.numpy as jnp

    # --- extract per-(i,t) argmax class from the packed keymax ---
    # low 12 bits hold (201-c) in units of 2^-24/ulp(keymax), exactly
    kb = np.ascontiguousarray(keymax).view(np.uint32)
    exp = ((kb >> 23) & 0xFF).astype(np.int64)
    low12 = (kb & 0xFFF).astype(np.int64)
    scale = np.exp2((exp - 127 - 23 + 24).astype(np.float64))
    cprime = np.rint(low12 * scale).astype(np.int64)
    idx = (NUM_CLASSES + 1) - cprime
    np.clip(idx, 0, NUM_CLASSES, out=idx)

    # --- cls_loss = sp_sum - scatter-subtract of gathered logits ---
    ii = np.arange(N_ROWS)[:, None]
    tt = np.arange(T)[None, :]
    g = lg[ii, idx, tt].astype(np.float64)
    cls_loss = sp_sum.astype(np.float64).copy()
    np.add.at(cls_loss, (ii, idx), -g)

    # --- last-snippet weight mask (exact reference semantics) ---
    lg99 = lg[:, :, T - 1]
    lb99 = lb[:, :, T - 1]
    labels99 = lb99.argmax(axis=1)
    is_bg = labels99 == NUM_CLASSES
    n_bg = int(is_bg.sum())

    cpu = jax.devices("cpu")[0]
    with jax.default_device(cpu):
        keys = jax.random.split(jax.random.key(42), T)
        k1, k2 = jax.random.split(keys[T - 1])
        u1 = np.asarray(jax.random.uniform(k1, (N_ROWS,)))
        u2 = np.asarray(jax.random.uniform(k2, (N_ROWS,)))
        score_mask = np.asarray(jax.nn.sigmoid(jnp.asarray(lg99))) >= np.float32(
            SCORE_THR
        )

    def _sel(u, m):
        um = np.where(is_bg, u, np.inf).astype(np.float32)
        order = np.argsort(um, kind="stable")
        ranks = np.zeros(N_ROWS, np.int64)
        ranks[order] = np.arange(N_ROWS)
        return is_bg & (ranks < m)

    sel_rare = _sel(u1, n_bg // 100)
    sel_common = _sel(u2, n_bg // 10)

    cls_id = np.arange(N_C)
    rare_m = (cls_id < 50).astype(np.float64)
    common_m = ((cls_id >= 50) & (cls_id < 150)).astype(np.float64)
    freq_m = ((cls_id >= 150) & (cls_id < 200)).astype(np.float64)
    bg_col = (cls_id == NUM_CLASSES).astype(np.float64)

    target99 = (labels99[:, None] == cls_id[None, :]).astype(np.float64)
    wm = np.where(is_bg[:, None], 0.0, score_mask.astype(np.float64))
    ind = (
        target99
        + is_bg[:, None] * (freq_m + bg_col)[None, :]
        + sel_rare[:, None] * rare_m[None, :]
        + sel_common[:, None] * common_m[None, :]
    )
    wm = np.maximum(wm, np.clip(ind, 0.0, 1.0))

    loss = (wm * cls_loss).sum() / (N_ROWS * T)
    return np.array(loss, dtype=np.float32)


def kernel(cls_logits_, labels_):
    lg = np.ascontiguousarray(np.asarray(cls_logits_, dtype=np.float32))
    lb = np.ascontiguousarray(np.asarray(labels_, dtype=np.float32))
    sp_sum, keymax, _ = run_device(lg, lb, trace=False)
    return _host_finalize(lg, lb, sp_sum, keymax)
